# revision 10
# baseline (speedup 1.0000x reference)
"""DirGCNConv (weighted gather + segment_sum + linear) on 8 Trainium2 NeuronCores.

v4: like v3 (4 SWDGE gather queues, dst tiles of 128), but the weighted
one-hot scatter matrices S are precomputed on the host and streamed from HBM
instead of being built per chunk on the DVE. This removes the per-chunk
tensor_scalar (was ~300 ns x 3.5k = 1.05 ms of DVE time) at the cost of
~113 MB/core of extra sequential HBM reads, and lets the chunk matmuls
pipeline back-to-back on TensorE.

  - Host: shard edges by dst range (12500/core); dst tiles of 128; edges
    grouped by (gather group of GT=3 tiles, src bank of 25000, tile), sorted
    by src, padded per (tile, bank) to 128-edge chunks (max over cores, one
    SPMD program). Streams: int16 bank-local gather indices (idx16), dense
    S[slot, 128] bf16 with S[slot, dl_slot] = w_slot (zeros on pad slots).
  - Device, per group: dma_gather x rows on SWDGE queue=bank (4 Q7 pairs in
    parallel); DMA the group's S blocks; per chunk j: TensorE PSUM
    agg.T[din, dst128] += G_j.T @ S_j; per tile: ACT copy PSUM -> agg
    staging [128, 384]; per group: out.T = W @ agg + b (matmul + ACT bias),
    one DMA out.
  - Host: out rows of core c = outT[:, :12500].T.
"""

import numpy as np
import ml_dtypes

bf16 = ml_dtypes.bfloat16

# problem constants (hardcoded per harness contract)
N_NODES = 100000
N_EDGES = 3200000
D = 128
NCORES = 8

# design constants
NLOC = N_NODES // NCORES      # 12500 dst nodes per core
TILE_W = 128                  # dst columns per tile
T_TILES = (NLOC + TILE_W - 1) // TILE_W   # 98
CHUNK = 128                   # edges per matmul chunk
NBANKS = 4
BROWS = N_NODES // NBANKS     # 25000 rows per src bank (int16-safe)
GT = 3                        # tiles per gather group / output supertile
NQUEUES = 4                   # SWDGE descriptor queues (Q7 core pairs)


def _host_prep(x, edge_index, edge_weight):
    """Vectorized host prep. Returns per-core streams + static block counts."""
    dst = np.asarray(edge_index[0], dtype=np.int64)
    src = np.asarray(edge_index[1], dtype=np.int64)
    w = np.asarray(edge_weight, dtype=np.float32)

    order = np.argsort(dst, kind="stable")
    dst_s, src_s, w_s = dst[order], src[order], w[order]
    bounds = np.searchsorted(dst_s, np.arange(NCORES + 1) * NLOC)

    cores = []
    counts_all = np.zeros((NCORES, T_TILES, NBANKS), np.int64)
    for c in range(NCORES):
        lo, hi = int(bounds[c]), int(bounds[c + 1])
        d_l = dst_s[lo:hi] - c * NLOC
        s_l = src_s[lo:hi]
        w_l = w_s[lo:hi]
        tile = d_l // TILE_W
        bank = s_l // BROWS
        grp = tile // GT
        # sort into segment-iteration order: (group, bank, tile), src minor
        o = np.lexsort((s_l, tile, bank, grp))
        tile, bank = tile[o], bank[o]
        cores.append((d_l[o], s_l[o], w_l[o], tile, bank))
        np.add.at(counts_all[c], (tile, bank), 1)

    # static padded counts per (tile, bank): max over cores, ceil to CHUNK
    P = ((counts_all.max(axis=0) + CHUNK - 1) // CHUNK) * CHUNK  # [T, NBANKS]
    nblk = P // CHUNK

    # segment-iteration order (group, bank, tile) -> slot offsets
    n_groups = (T_TILES + GT - 1) // GT
    seg_order = []  # (t, b) in iteration order
    for g in range(n_groups):
        for b in range(NBANKS):
            for t in range(g * GT, min((g + 1) * GT, T_TILES)):
                seg_order.append((t, b))
    seg_sizes = np.array([P[t, b] for t, b in seg_order], np.int64)
    seg_starts = np.concatenate([[0], np.cumsum(seg_sizes)])
    tot_slots = int(seg_starts[-1])
    totblk = tot_slots // CHUNK
    # map (t, b) -> slot start
    seg_start_tb = np.zeros((T_TILES, NBANKS), np.int64)
    for i, (t, b) in enumerate(seg_order):
        seg_start_tb[t, b] = seg_starts[i]

    # per-group metadata for the program builder
    groups = []
    for g in range(n_groups):
        tiles_g = list(range(g * GT, min((g + 1) * GT, T_TILES)))
        g_slot0 = int(seg_start_tb[tiles_g[0], 0])
        g_blk0 = g_slot0 // CHUNK
        bank_segs = []  # (local block offset, num blocks) per bank
        for b in range(NBANKS):
            s0 = int(seg_start_tb[tiles_g[0], b])
            ln = int(sum(P[t, b] for t in tiles_g))
            bank_segs.append(((s0 - g_slot0) // CHUNK, ln // CHUNK))
        tile_blocks = []  # per tile: list of local block indices (group-rel)
        for t in tiles_g:
            blks = []
            for b in range(NBANKS):
                s0 = int(seg_start_tb[t, b])
                blks.extend(range((s0 - g_slot0) // CHUNK,
                                  (s0 - g_slot0) // CHUNK + int(nblk[t, b])))
            tile_blocks.append((t, blks))
        nblk_g = int(sum(P[t, b] for t in tiles_g for b in range(NBANKS))) // CHUNK
        groups.append({
            "blk0": g_blk0, "nblk": nblk_g,
            "bank_segs": bank_segs, "tile_blocks": tile_blocks,
        })

    # per-core streams
    per_core = []
    for c in range(NCORES):
        d_l, s_l, w_l, tile, bank = cores[c]
        ne = len(d_l)
        # edges are sorted in segment-iteration order; compute positions
        cnt_iter = np.array(
            [counts_all[c, t, b] for t, b in seg_order], np.int64)
        first = np.concatenate([[0], np.cumsum(cnt_iter)])[:-1]
        pos = (np.repeat(seg_starts[:-1], cnt_iter)
               + np.arange(ne) - np.repeat(first, cnt_iter))

        # pad slots repeat the segment's last real src row (HBM row stays
        # hot); their S row is all-zero so they contribute nothing.
        idx_stream = np.zeros(tot_slots, np.int16)
        idx_stream[pos] = (s_l % BROWS).astype(np.int16)
        valid = np.zeros(tot_slots, bool)
        valid[pos] = True
        vi = np.where(valid, np.arange(tot_slots), 0)
        np.maximum.accumulate(vi, out=vi)
        idx_stream = idx_stream[vi]

        idx16 = np.ascontiguousarray(
            np.tile(idx_stream.reshape(-1, 16).T, (8, 1)))   # [128, totblk*8]

        # dense weighted one-hot stream: S[slot, dl_slot] = w_slot
        s_flat = np.zeros((tot_slots, TILE_W), np.float32)
        s_flat[pos, (d_l % TILE_W)] = w_l
        s_np = np.ascontiguousarray(
            s_flat.astype(bf16).reshape(totblk, CHUNK, TILE_W)
            .transpose(1, 0, 2))                             # [128, totblk, T]
        per_core.append({"idx16": idx16, "s": s_np})

    xb = np.asarray(x, np.float32).astype(bf16)  # [N, D]
    banks = [np.ascontiguousarray(xb[b * BROWS:(b + 1) * BROWS])
             for b in range(NBANKS)]

    return per_core, banks, groups, totblk


def _build_program(groups, totblk):
    import concourse.bass as bass  # noqa: F401
    import concourse.bacc as bacc
    import concourse.mybir as mybir
    import concourse.tile as tile
    from concourse import library_config

    nc = bacc.Bacc("TRN2", target_bir_lowering=False, debug=False,
                   num_devices=NCORES, num_swdge_queues=NQUEUES)

    xb_d = [nc.dram_tensor(f"xb{b}", [BROWS, D], mybir.dt.bfloat16,
                           kind="ExternalInput") for b in range(NBANKS)]
    idx_d = nc.dram_tensor("idx16", [128, totblk * 8], mybir.dt.int16,
                           kind="ExternalInput")
    s_d = nc.dram_tensor("s", [128, totblk, TILE_W], mybir.dt.bfloat16,
                         kind="ExternalInput")
    wt_d = nc.dram_tensor("wt", [D, D], mybir.dt.float32, kind="ExternalInput")
    b_d = nc.dram_tensor("b", [D, 1], mybir.dt.float32, kind="ExternalInput")
    out_d = nc.dram_tensor("outT", [D, T_TILES * TILE_W], mybir.dt.float32,
                           kind="ExternalOutput")

    max_nblk = max(g["nblk"] for g in groups)
    SUP_W = GT * TILE_W  # supertile width (384)

    with tile.TileContext(nc) as tc:
        with (
            tc.tile_pool(name="const", bufs=1) as constp,
            tc.tile_pool(name="meta", bufs=3) as metap,
            tc.tile_pool(name="gather", bufs=3) as gatherp,
            tc.tile_pool(name="s", bufs=2) as sp,
            tc.tile_pool(name="agg", bufs=2) as aggp,
            tc.tile_pool(name="outp", bufs=2) as outp,
            tc.tile_pool(name="psum", bufs=4, space="PSUM") as psump,
            tc.tile_pool(name="psum2", bufs=2, space="PSUM") as psum2p,
        ):
            nc.gpsimd.load_library(library_config.mlp)

            wt_t = constp.tile([D, D], mybir.dt.float32)
            b_t = constp.tile([D, 1], mybir.dt.float32)
            nc.sync.dma_start(wt_t[:], wt_d[:])
            nc.sync.dma_start(b_t[:], b_d[:])

            for gi, g in enumerate(groups):
                blk0, nblk_g = g["blk0"], g["nblk"]
                ntile_g = len(g["tile_blocks"])
                sup_w = ntile_g * TILE_W
                idx_t = metap.tile([128, max_nblk * 8], mybir.dt.int16,
                                   tag="idx")
                nc.sync.dma_start(idx_t[:, :nblk_g * 8],
                                  idx_d[:, blk0 * 8:(blk0 + nblk_g) * 8])

                s_t = sp.tile([128, max_nblk, TILE_W], mybir.dt.bfloat16,
                              tag="s")
                nc.sync.dma_start(s_t[:, :nblk_g, :],
                                  s_d[:, blk0:blk0 + nblk_g, :])

                g_t = gatherp.tile([128, max_nblk, D], mybir.dt.bfloat16,
                                   tag="g")
                # One SWDGE ring holds 1024 descriptors, so one gather call
                # covers up to 8 blocks of 128 rows. Bank b's calls go to
                # SWDGE queue b (4 Q7 core pairs in parallel), issued
                # bank-interleaved so the 8-deep GpSimd engine queue always
                # spans all 4 queues. GpSimd retires in order, so each
                # 4-queue "wave" advances at the pace of its largest call:
                # split every bank into the same number of near-equal calls
                # so no wave is ragged. Calls are capped at 4 blocks (~512
                # descriptors) so two calls fit in the 1024-desc ring and a
                # call's descriptor generation overlaps the previous call's
                # SDMA drain.
                MAXG = 4
                ncalls = max((g["bank_segs"][b][1] + MAXG - 1) // MAXG
                             for b in range(NBANKS))
                call_lists = []  # per bank: list of (a, ln)
                for b in range(NBANKS):
                    boff, blen = g["bank_segs"][b]
                    base, rem = divmod(blen, ncalls)
                    calls = []
                    a = boff
                    for ci in range(ncalls):
                        ln = base + (1 if ci < rem else 0)
                        if ln:
                            calls.append((a, ln))
                            a += ln
                    call_lists.append(calls)
                for ci in range(ncalls):
                    for b in range(NBANKS):
                        if ci >= len(call_lists[b]):
                            continue
                        a, ln = call_lists[b][ci]
                        nc.gpsimd.dma_gather(
                            g_t[:, a:a + ln, :],
                            xb_d[b][:],
                            idx_t[:, a * 8:(a + ln) * 8],
                            ln * CHUNK,
                            ln * CHUNK,
                            D,
                            queue_num=b,
                        )

                agg4_t = aggp.tile([D, SUP_W], mybir.dt.float32, tag="agg")
                for ti, (t, blks) in enumerate(g["tile_blocks"]):
                    assert blks, f"tile {t} has no edge blocks"
                    psum_t = psump.tile([D, TILE_W], mybir.dt.float32,
                                        tag="p1")
                    for k, j in enumerate(blks):
                        nc.tensor.matmul(
                            psum_t[:], g_t[:, j, :], s_t[:, j, :],
                            start=(k == 0), stop=(k == len(blks) - 1),
                        )

                    nc.scalar.copy(
                        agg4_t[:, ti * TILE_W:(ti + 1) * TILE_W], psum_t[:])

                psum2_t = psum2p.tile([D, SUP_W], mybir.dt.float32, tag="p2")
                nc.tensor.matmul(psum2_t[:, :sup_w], wt_t[:],
                                 agg4_t[:, :sup_w], start=True, stop=True)

                out_t = outp.tile([D, SUP_W], mybir.dt.float32, tag="o")
                nc.scalar.activation(
                    out_t[:, :sup_w], psum2_t[:, :sup_w],
                    mybir.ActivationFunctionType.Identity,
                    bias=b_t[:, 0:1], scale=1.0,
                )
                t0 = g["tile_blocks"][0][0]
                nc.sync.dma_start(
                    out_d[:, t0 * TILE_W:t0 * TILE_W + sup_w],
                    out_t[:, :sup_w])

    nc.compile()
    return nc


LAST_RES = None


def kernel(x, edge_index, edge_weight, W, b):
    import os
    from concourse.bass_utils import run_bass_kernel_spmd

    per_core, banks, groups, totblk = _host_prep(x, edge_index, edge_weight)

    nc = _build_program(groups, totblk)

    WT = np.ascontiguousarray(np.asarray(W, np.float32).T)  # [din, dout]
    bcol = np.ascontiguousarray(np.asarray(b, np.float32).reshape(D, 1))

    in_maps = []
    for c in range(NCORES):
        p = per_core[c]
        m = {f"xb{i}": banks[i] for i in range(NBANKS)}
        m.update({
            "idx16": p["idx16"], "s": p["s"], "wt": WT, "b": bcol,
        })
        in_maps.append(m)

    res = run_bass_kernel_spmd(
        nc, in_maps, core_ids=list(range(NCORES)),
        trace=bool(int(os.environ.get("KERNEL_TRACE", "0"))),
    )
    global LAST_RES
    LAST_RES = res

    out = np.empty((N_NODES, D), np.float32)
    for c in range(NCORES):
        outT = res.results[c]["outT"]  # [D, T*TILE_W]
        out[c * NLOC:(c + 1) * NLOC] = outT[:, :NLOC].T
    return out


if __name__ == "__main__":
    # smoke test with random data (self-contained)
    rng = np.random.default_rng(0)
    x = rng.standard_normal((N_NODES, D)).astype(np.float32)
    ei = rng.integers(0, N_NODES, size=(2, N_EDGES)).astype(np.int64)
    ew = rng.random(N_EDGES).astype(np.float32)
    W = (rng.standard_normal((D, D)) / np.sqrt(D)).astype(np.float32)
    b = (rng.standard_normal(D) * 0.01).astype(np.float32)
    out = kernel(x, ei, ew, W, b)
    print("out", out.shape, out.dtype)


# revision 40
# speedup vs baseline: 1.1404x; 1.1404x over previous
"""DirGCNConv (weighted gather + segment_sum + linear) on 8 Trainium2 NeuronCores.

Computation (reference):
    dst, src = edge_index
    agg[d] = sum_{e: dst_e == d} edge_weight[e] * x[src_e]     # [N, D]
    out = agg @ W.T + b

Strategy (dst-sharded, no collectives):
  - Host: shard edges by dst range (12500/core); dst tiles of 128 columns;
    edges grouped by (group of GT=3 tiles, src bank of 25000, tile), sorted
    by src. Slot layout: within each (group, bank), tile regions are packed
    at 16-slot granularity (static size = max edge count over the 8 cores,
    rounded to 16) and only the (group, bank) total is padded to 128-slot
    blocks, so tiles share boundary blocks instead of each wasting most of
    one. Pad slots re-gather the previous real row (HBM row stays hot) and
    have all-zero S rows.
  - Streams: int16 bank-local gather indices (idx16, gather ucode layout),
    and host-precomputed weighted one-hot S blocks [128 slots, 128 dst]
    bf16, one S block per (tile, covered gather block) with foreign-tile
    slots zeroed -- shared boundary blocks appear once per adjacent tile.
  - Device, per group: dma_gather x rows from the 4 bf16 bank tables on
    SWDGE queue=bank (descriptor generation runs on all 4 Q7 core pairs
    concurrently; calls are <=8 blocks to fit the 1024-descriptor ring,
    issued bank-interleaved in equal-size waves because GpSimd retires in
    order); DMA the group's S blocks; per (tile, block): TensorE PSUM
    agg.T[din, dst128] += G_blk.T @ S_blk; per tile: ACT copy PSUM -> agg
    staging; per group: out.T = W @ agg + b (one fp32 matmul + ACT bias),
    one DMA out.
  - Host: out rows of core c = outT[:, :12500].T (column = local node id).
"""

import numpy as np
import ml_dtypes

bf16 = ml_dtypes.bfloat16

# problem constants (hardcoded per harness contract)
N_NODES = 100000
N_EDGES = 3200000
D = 128
NCORES = 8

# design constants
NLOC = N_NODES // NCORES      # 12500 dst nodes per core
TILE_W = 128                  # dst columns per tile
T_TILES = (NLOC + TILE_W - 1) // TILE_W   # 98
CHUNK = 128                   # slots per gather block / matmul chunk
NBANKS = 4
BROWS = N_NODES // NBANKS     # 25000 rows per src bank (int16-safe)
GT = 3                        # tiles per gather group / output supertile
NQUEUES = 4                   # SWDGE descriptor queues (Q7 core pairs)
ALIGN = 1                     # tile-region slot alignment


def _host_prep(x, edge_index, edge_weight):
    """Vectorized host prep. Returns per-core streams + static layout."""
    dst = np.asarray(edge_index[0], dtype=np.int64)
    src = np.asarray(edge_index[1], dtype=np.int64)
    w = np.asarray(edge_weight, dtype=np.float32)

    # Degree-balanced dst assignment: static region sizes are max-over-cores
    # of per-(tile, bank) edge counts, so stripe dst nodes by in-degree
    # across cores (equalizes core totals, worst core was +4%) and across
    # tiles within a core (equalizes tile totals). The host un-permutes the
    # output columns at the end.
    deg = np.bincount(dst, minlength=N_NODES)
    dorder = np.argsort(deg, kind="stable")
    core_of = np.empty(N_NODES, np.int64)
    rank = np.empty(N_NODES, np.int64)
    core_of[dorder] = np.arange(N_NODES) % NCORES
    rank[dorder] = np.arange(N_NODES) // NCORES       # 0..NLOC-1 per core
    tile_of = rank % T_TILES
    pos_of = rank // T_TILES                          # dst column in tile
    col_of = tile_of * TILE_W + pos_of                # outT column

    # First group holds a single tile so the pipeline ramps quickly (the
    # first MMs start after ~1/3 the gather work); later groups hold GT.
    group_tiles = [[0]]
    tt = 1
    while tt < T_TILES:
        group_tiles.append(list(range(tt, min(tt + GT, T_TILES))))
        tt += GT
    n_groups = len(group_tiles)
    grp_of_tile = np.empty(T_TILES, np.int64)
    for gi, ts in enumerate(group_tiles):
        grp_of_tile[ts] = gi

    core_e = core_of[dst]
    eorder = np.argsort(core_e, kind="stable")
    ebounds = np.searchsorted(core_e[eorder], np.arange(NCORES + 1))

    cores = []
    counts_all = np.zeros((NCORES, T_TILES, NBANKS), np.int64)
    for c in range(NCORES):
        sel = eorder[int(ebounds[c]):int(ebounds[c + 1])]
        d_g = dst[sel]
        s_l = src[sel]
        w_l = w[sel]
        tile = tile_of[d_g]
        dl = pos_of[d_g]
        bank = s_l // BROWS
        grp = grp_of_tile[tile]
        # sort into segment-iteration order: (group, bank, tile), src minor
        o = np.lexsort((s_l, tile, bank, grp))
        tile, bank = tile[o], bank[o]
        cores.append((dl[o], s_l[o], w_l[o], tile, bank))
        np.add.at(counts_all[c], (tile, bank), 1)

    # static tile-region sizes: max edge count over cores, rounded to 16
    S_tb = ((counts_all.max(axis=0) + ALIGN - 1) // ALIGN) * ALIGN  # [T, NB]

    # G-slot layout: (group, bank) ranges in iteration order, each padded to
    # a multiple of 128; tile regions packed back-to-back inside.
    R_tb = np.zeros((T_TILES, NBANKS), np.int64)     # global slot offset
    gb_blk0 = np.zeros((n_groups, NBANKS), np.int64)  # first G-block
    gb_nblk = np.zeros((n_groups, NBANKS), np.int64)
    slot = 0
    for g in range(n_groups):
        for b in range(NBANKS):
            gb_blk0[g, b] = slot // CHUNK
            for t in group_tiles[g]:
                R_tb[t, b] = slot
                slot += int(S_tb[t, b])
            slot = ((slot + CHUNK - 1) // CHUNK) * CHUNK
            gb_nblk[g, b] = slot // CHUNK - gb_blk0[g, b]
    tot_slots = slot
    totblk = tot_slots // CHUNK

    # S-block layout: per group, per (bank, tile): one S block per G-block
    # the tile's region touches. Shared boundary G-blocks get one S block
    # per adjacent tile (foreign slots zeroed).
    # smap entries: (t, b, jG) -> jS  plus per-group metadata.
    groups = []
    s_entries = []  # global list: (jS, jG, t, b)
    jS = 0
    for g in range(n_groups):
        g_blk0 = int(gb_blk0[g, 0])
        nblk_g = int(gb_nblk[g].sum())
        sblk0 = jS
        tile_mms = []   # per tile: list of (jG_local, jS_local)
        for t in group_tiles[g]:
            mms = []
            for b in range(NBANKS):
                lo_blk = int(R_tb[t, b]) // CHUNK
                hi_blk = (int(R_tb[t, b]) + int(S_tb[t, b]) - 1) // CHUNK
                for jG in range(lo_blk, hi_blk + 1):
                    mms.append((jG - g_blk0, jS - sblk0))
                    s_entries.append((jS, jG, t, b))
                    jS += 1
            tile_mms.append((t, mms))
        groups.append({
            "blk0": g_blk0, "nblk": nblk_g,
            "sblk0": sblk0, "nsblk": jS - sblk0,
            "bank_segs": [(int(gb_blk0[g, b] - gb_blk0[g, 0]),
                           int(gb_nblk[g, b])) for b in range(NBANKS)],
            "tile_mms": tile_mms,
        })
    tot_sblk = jS

    # per-core streams
    per_core = []
    for c in range(NCORES):
        d_l, s_l, w_l, tile, bank = cores[c]
        ne = len(d_l)
        # per-edge slot: R_tb[t, b] + rank within its (t, b) segment.
        # edges are already sorted in segment-iteration order.
        seg_sizes_iter = []
        seg_R_iter = []
        for g in range(n_groups):
            for b in range(NBANKS):
                for t in group_tiles[g]:
                    seg_sizes_iter.append(counts_all[c, t, b])
                    seg_R_iter.append(R_tb[t, b])
        seg_sizes_iter = np.array(seg_sizes_iter, np.int64)
        seg_R_iter = np.array(seg_R_iter, np.int64)
        first = np.concatenate([[0], np.cumsum(seg_sizes_iter)])[:-1]
        pos = (np.repeat(seg_R_iter, seg_sizes_iter)
               + np.arange(ne) - np.repeat(first, seg_sizes_iter))

        # gather idx stream: real rows at their slots; pad slots repeat the
        # previous real row (forward fill).
        idx_stream = np.zeros(tot_slots, np.int16)
        idx_stream[pos] = (s_l % BROWS).astype(np.int16)
        valid = np.zeros(tot_slots, bool)
        valid[pos] = True
        vi = np.where(valid, np.arange(tot_slots), 0)
        np.maximum.accumulate(vi, out=vi)
        idx_stream = idx_stream[vi]

        idx16 = np.ascontiguousarray(
            np.tile(idx_stream.reshape(-1, 16).T, (8, 1)))   # [128, totblk*8]

        # S stream: s_flat[jS*128 + lane, dl] = w for edges whose G-slot
        # falls in (t, b, jG); pad/foreign slots stay zero.
        jG_e = pos // CHUNK
        lane_e = pos % CHUNK
        # map (t, b, jG) -> jS: build lookup per (t, b) over its block span
        jS_e = np.empty(ne, np.int64)
        # vectorized via dict over segments (396 entries)
        tb_of_edge = tile * NBANKS + bank
        s_lut = {}
        for (jSv, jGv, tv, bv) in s_entries:
            s_lut[(tv * NBANKS + bv, jGv)] = jSv
        # per-edge lookup: loop over segments, slice by sorted order
        seg_ids = tb_of_edge
        # edges grouped by segment contiguously; find boundaries
        segb = np.flatnonzero(np.diff(seg_ids)) + 1
        starts = np.concatenate([[0], segb])
        ends = np.concatenate([segb, [ne]])
        for s0, e0 in zip(starts, ends):
            key = int(seg_ids[s0])
            jSs = np.array([s_lut[(key, int(jg))] for jg in
                            np.unique(jG_e[s0:e0])], np.int64)
            uj, inv = np.unique(jG_e[s0:e0], return_inverse=True)
            jS_e[s0:e0] = jSs[inv]

        s_flat = np.zeros((tot_sblk * CHUNK, TILE_W), np.float32)
        s_flat[jS_e * CHUNK + lane_e, d_l] = w_l
        s_np = np.ascontiguousarray(
            s_flat.astype(bf16).reshape(tot_sblk, CHUNK, TILE_W)
            .transpose(1, 0, 2))                             # [128, sblk, T]
        per_core.append({"idx16": idx16, "s": s_np})

    xb = np.asarray(x, np.float32).astype(bf16)  # [N, D]
    banks = [np.ascontiguousarray(xb[b * BROWS:(b + 1) * BROWS])
             for b in range(NBANKS)]

    return per_core, banks, groups, totblk, tot_sblk, core_of, col_of


def _build_program(groups, totblk, tot_sblk):
    import concourse.bass as bass  # noqa: F401
    import concourse.bacc as bacc
    import concourse.mybir as mybir
    import concourse.tile as tile
    from concourse import library_config

    nc = bacc.Bacc("TRN2", target_bir_lowering=False, debug=False,
                   num_devices=NCORES, num_swdge_queues=NQUEUES)

    xb_d = [nc.dram_tensor(f"xb{b}", [BROWS, D], mybir.dt.bfloat16,
                           kind="ExternalInput") for b in range(NBANKS)]
    idx_d = nc.dram_tensor("idx16", [128, totblk * 8], mybir.dt.int16,
                           kind="ExternalInput")
    s_d = nc.dram_tensor("s", [128, tot_sblk, TILE_W], mybir.dt.bfloat16,
                         kind="ExternalInput")
    wt_d = nc.dram_tensor("wt", [D, D], mybir.dt.float32, kind="ExternalInput")
    b_d = nc.dram_tensor("b", [D, 1], mybir.dt.float32, kind="ExternalInput")
    out_d = nc.dram_tensor("outT", [D, T_TILES * TILE_W], mybir.dt.float32,
                           kind="ExternalOutput")

    max_nblk = max(g["nblk"] for g in groups)
    max_nsblk = max(g["nsblk"] for g in groups)
    SUP_W = GT * TILE_W  # supertile width (384)

    with tile.TileContext(nc) as tc:
        with (
            tc.tile_pool(name="const", bufs=1) as constp,
            tc.tile_pool(name="meta", bufs=6) as metap,
            tc.tile_pool(name="gather", bufs=3) as gatherp,
            tc.tile_pool(name="s", bufs=2) as sp,
            tc.tile_pool(name="agg", bufs=2) as aggp,
            tc.tile_pool(name="outp", bufs=2) as outp,
            tc.tile_pool(name="psum", bufs=6, space="PSUM") as psump,
            tc.tile_pool(name="psum2", bufs=2, space="PSUM") as psum2p,
        ):
            nc.gpsimd.load_library(library_config.mlp)

            wt_t = constp.tile([D, D], mybir.dt.float32)
            b_t = constp.tile([D, 1], mybir.dt.float32)
            nc.sync.dma_start(wt_t[:], wt_d[:])
            nc.sync.dma_start(b_t[:], b_d[:])

            for gi, g in enumerate(groups):
                blk0, nblk_g = g["blk0"], g["nblk"]
                sblk0, nsblk_g = g["sblk0"], g["nsblk"]
                ntile_g = len(g["tile_mms"])
                sup_w = ntile_g * TILE_W
                idx_t = metap.tile([128, max_nblk * 8], mybir.dt.int16,
                                   tag="idx")
                nc.sync.dma_start(idx_t[:, :nblk_g * 8],
                                  idx_d[:, blk0 * 8:(blk0 + nblk_g) * 8])

                s_t = sp.tile([128, max_nsblk, TILE_W], mybir.dt.bfloat16,
                              tag="s")
                nc.sync.dma_start(s_t[:, :nsblk_g, :],
                                  s_d[:, sblk0:sblk0 + nsblk_g, :])

                g_t = gatherp.tile([128, max_nblk, D], mybir.dt.bfloat16,
                                   tag="g")
                # One gather call is capped at 1024 indices (8 blocks) --
                # larger num_idxs hangs the ucode on hardware. Bank b's
                # calls go to SWDGE queue b (4 Q7 core pairs in parallel),
                # issued bank-interleaved so the 8-deep GpSimd engine queue
                # always spans all 4 queues. GpSimd retires in order, so
                # each 4-queue "wave" advances at the pace of its largest
                # call: split every bank into the same number of near-equal
                # calls so no wave is ragged.
                MAXG = 8
                ncalls = max((g["bank_segs"][b][1] + MAXG - 1) // MAXG
                             for b in range(NBANKS))
                # Full 8-block waves first, remainder last: waves stay
                # bank-balanced, and the group's final (small) wave drains
                # quickly so its G buffer frees sooner for group g+2.
                call_lists = []  # per bank: list of (a, ln)
                for b in range(NBANKS):
                    boff, blen = g["bank_segs"][b]
                    calls = []
                    a = boff
                    while blen > 0:
                        ln = min(MAXG, blen)
                        calls.append((a, ln))
                        a += ln
                        blen -= ln
                    call_lists.append(calls)
                for ci in range(ncalls):
                    for b in range(NBANKS):
                        if ci >= len(call_lists[b]):
                            continue
                        a, ln = call_lists[b][ci]
                        nc.gpsimd.dma_gather(
                            g_t[:, a:a + ln, :],
                            xb_d[b][:],
                            idx_t[:, a * 8:(a + ln) * 8],
                            ln * CHUNK,
                            ln * CHUNK,
                            D,
                            queue_num=b,
                        )

                agg4_t = aggp.tile([D, SUP_W], mybir.dt.float32, tag="agg")
                for ti, (t, mms) in enumerate(g["tile_mms"]):
                    assert mms, f"tile {t} has no edge blocks"
                    psum_t = psump.tile([D, TILE_W], mybir.dt.float32,
                                        tag="p1")
                    for k, (jG, jS) in enumerate(mms):
                        nc.tensor.matmul(
                            psum_t[:], g_t[:, jG, :], s_t[:, jS, :],
                            start=(k == 0), stop=(k == len(mms) - 1),
                        )

                    nc.scalar.copy(
                        agg4_t[:, ti * TILE_W:(ti + 1) * TILE_W], psum_t[:])

                psum2_t = psum2p.tile([D, SUP_W], mybir.dt.float32, tag="p2")
                nc.tensor.matmul(psum2_t[:, :sup_w], wt_t[:],
                                 agg4_t[:, :sup_w], start=True, stop=True)

                out_t = outp.tile([D, SUP_W], mybir.dt.float32, tag="o")
                nc.scalar.activation(
                    out_t[:, :sup_w], psum2_t[:, :sup_w],
                    mybir.ActivationFunctionType.Identity,
                    bias=b_t[:, 0:1], scale=1.0,
                )
                t0 = g["tile_mms"][0][0]
                nc.sync.dma_start(
                    out_d[:, t0 * TILE_W:t0 * TILE_W + sup_w],
                    out_t[:, :sup_w])

    nc.compile()
    return nc


LAST_RES = None


def kernel(x, edge_index, edge_weight, W, b):
    import os
    from concourse.bass_utils import run_bass_kernel_spmd

    per_core, banks, groups, totblk, tot_sblk, core_of, col_of = _host_prep(
        x, edge_index, edge_weight)

    nc = _build_program(groups, totblk, tot_sblk)

    WT = np.ascontiguousarray(np.asarray(W, np.float32).T)  # [din, dout]
    bcol = np.ascontiguousarray(np.asarray(b, np.float32).reshape(D, 1))

    in_maps = []
    for c in range(NCORES):
        p = per_core[c]
        m = {f"xb{i}": banks[i] for i in range(NBANKS)}
        m.update({
            "idx16": p["idx16"], "s": p["s"], "wt": WT, "b": bcol,
        })
        in_maps.append(m)

    res = run_bass_kernel_spmd(
        nc, in_maps, core_ids=list(range(NCORES)),
        trace=bool(int(os.environ.get("KERNEL_TRACE", "0"))),
    )
    global LAST_RES
    LAST_RES = res

    out = np.empty((N_NODES, D), np.float32)
    for c in range(NCORES):
        outT = res.results[c]["outT"]  # [D, T*TILE_W]
        dsts_c = np.flatnonzero(core_of == c)
        out[dsts_c] = outT[:, col_of[dsts_c]].T
    return out


if __name__ == "__main__":
    # smoke test with random data (self-contained)
    rng = np.random.default_rng(0)
    x = rng.standard_normal((N_NODES, D)).astype(np.float32)
    ei = rng.integers(0, N_NODES, size=(2, N_EDGES)).astype(np.int64)
    ew = rng.random(N_EDGES).astype(np.float32)
    W = (rng.standard_normal((D, D)) / np.sqrt(D)).astype(np.float32)
    b = (rng.standard_normal(D) * 0.01).astype(np.float32)
    out = kernel(x, ei, ew, W, b)
    print("out", out.shape, out.dtype)


# revision 43
# speedup vs baseline: 1.2361x; 1.0839x over previous
"""DirGCNConv (weighted gather + segment_sum + linear) on 8 Trainium2 NeuronCores.

Computation (reference):
    dst, src = edge_index
    agg[d] = sum_{e: dst_e == d} edge_weight[e] * x[src_e]     # [N, D]
    out = agg @ W.T + b

Strategy (dst-sharded, no collectives):
  - Host: shard edges by dst range (12500/core); dst tiles of 128 columns;
    edges grouped by (group of GT=3 tiles, src bank of 25000, tile), sorted
    by src. Slot layout: within each (group, bank), tile regions are packed
    at 16-slot granularity (static size = max edge count over the 8 cores,
    rounded to 16) and only the (group, bank) total is padded to 128-slot
    blocks, so tiles share boundary blocks instead of each wasting most of
    one. Pad slots re-gather the previous real row (HBM row stays hot) and
    have all-zero S rows.
  - Streams: int16 bank-local gather indices (idx16, gather ucode layout),
    and host-precomputed weighted one-hot S blocks [128 slots, 128 dst]
    bf16, one S block per (tile, covered gather block) with foreign-tile
    slots zeroed -- shared boundary blocks appear once per adjacent tile.
  - Device, per group: dma_gather x rows from the 4 bf16 bank tables on
    SWDGE queue=bank (descriptor generation runs on all 4 Q7 core pairs
    concurrently; calls are <=8 blocks to fit the 1024-descriptor ring,
    issued bank-interleaved in equal-size waves because GpSimd retires in
    order); DMA the group's S blocks; per (tile, block): TensorE PSUM
    agg.T[din, dst128] += G_blk.T @ S_blk; per tile: ACT copy PSUM -> agg
    staging; per group: out.T = W @ agg + b (one fp32 matmul + ACT bias),
    one DMA out.
  - Host: out rows of core c = outT[:, :12500].T (column = local node id).
"""

import numpy as np
import ml_dtypes

bf16 = ml_dtypes.bfloat16

# problem constants (hardcoded per harness contract)
N_NODES = 100000
N_EDGES = 3200000
D = 128
NCORES = 8

# design constants
NLOC = N_NODES // NCORES      # 12500 dst nodes per core
TILE_W = 128                  # dst columns per tile
T_TILES = (NLOC + TILE_W - 1) // TILE_W   # 98
CHUNK = 128                   # slots per gather block / matmul chunk
NBANKS = 4
BROWS = N_NODES // NBANKS     # 25000 rows per src bank (int16-safe)
GT = 3                        # tiles per gather group / output supertile
NQUEUES = 4                   # SWDGE descriptor queues (Q7 core pairs)
ALIGN = 1                     # tile-region slot alignment


def _host_prep(x, edge_index, edge_weight):
    """Vectorized host prep. Returns per-core streams + static layout."""
    dst = np.asarray(edge_index[0], dtype=np.int64)
    src = np.asarray(edge_index[1], dtype=np.int64)
    w = np.asarray(edge_weight, dtype=np.float32)

    # Degree-balanced dst assignment: static region sizes are max-over-cores
    # of per-(tile, bank) edge counts, so stripe dst nodes by in-degree
    # across cores (equalizes core totals, worst core was +4%) and across
    # tiles within a core (equalizes tile totals). The host un-permutes the
    # output columns at the end.
    deg = np.bincount(dst, minlength=N_NODES)
    dorder = np.argsort(deg, kind="stable")
    core_of = np.empty(N_NODES, np.int64)
    rank = np.empty(N_NODES, np.int64)
    core_of[dorder] = np.arange(N_NODES) % NCORES
    rank[dorder] = np.arange(N_NODES) // NCORES       # 0..NLOC-1 per core
    tile_of = rank % T_TILES
    pos_of = rank // T_TILES                          # dst column in tile
    col_of = tile_of * TILE_W + pos_of                # outT column

    core_e = core_of[dst]
    eorder = np.argsort(core_e, kind="stable")
    ebounds = np.searchsorted(core_e[eorder], np.arange(NCORES + 1))

    cores = []
    counts_all = np.zeros((NCORES, T_TILES, NBANKS), np.int64)
    for c in range(NCORES):
        sel = eorder[int(ebounds[c]):int(ebounds[c + 1])]
        d_g = dst[sel]
        s_l = src[sel]
        w_l = w[sel]
        tile = tile_of[d_g]
        dl = pos_of[d_g]
        bank = s_l // BROWS
        grp = tile // GT
        # sort into segment-iteration order: (group, bank, tile), src minor
        o = np.lexsort((s_l, tile, bank, grp))
        tile, bank = tile[o], bank[o]
        cores.append((dl[o], s_l[o], w_l[o], tile, bank))
        np.add.at(counts_all[c], (tile, bank), 1)

    # static tile-region sizes: max edge count over cores, rounded to 16
    S_tb = ((counts_all.max(axis=0) + ALIGN - 1) // ALIGN) * ALIGN  # [T, NB]

    n_groups = (T_TILES + GT - 1) // GT
    group_tiles = [list(range(g * GT, min((g + 1) * GT, T_TILES)))
                   for g in range(n_groups)]

    # G-slot layout: (group, bank) ranges in iteration order, each padded to
    # a multiple of 128; tile regions packed back-to-back inside.
    R_tb = np.zeros((T_TILES, NBANKS), np.int64)     # global slot offset
    gb_blk0 = np.zeros((n_groups, NBANKS), np.int64)  # first G-block
    gb_nblk = np.zeros((n_groups, NBANKS), np.int64)
    slot = 0
    for g in range(n_groups):
        for b in range(NBANKS):
            gb_blk0[g, b] = slot // CHUNK
            for t in group_tiles[g]:
                R_tb[t, b] = slot
                slot += int(S_tb[t, b])
            slot = ((slot + CHUNK - 1) // CHUNK) * CHUNK
            gb_nblk[g, b] = slot // CHUNK - gb_blk0[g, b]
    tot_slots = slot
    totblk = tot_slots // CHUNK

    # S-block layout: per group, per (bank, tile): one S block per G-block
    # the tile's region touches. Shared boundary G-blocks get one S block
    # per adjacent tile (foreign slots zeroed).
    # smap entries: (t, b, jG) -> jS  plus per-group metadata.
    groups = []
    s_entries = []  # global list: (jS, jG, t, b)
    jS = 0
    for g in range(n_groups):
        g_blk0 = int(gb_blk0[g, 0])
        nblk_g = int(gb_nblk[g].sum())
        sblk0 = jS
        tile_mms = []   # per tile: list of (jG_local, jS_local)
        for t in group_tiles[g]:
            mms = []
            for b in range(NBANKS):
                lo_blk = int(R_tb[t, b]) // CHUNK
                hi_blk = (int(R_tb[t, b]) + int(S_tb[t, b]) - 1) // CHUNK
                for jG in range(lo_blk, hi_blk + 1):
                    mms.append((jG - g_blk0, jS - sblk0))
                    s_entries.append((jS, jG, t, b))
                    jS += 1
            tile_mms.append((t, mms))
        groups.append({
            "blk0": g_blk0, "nblk": nblk_g,
            "sblk0": sblk0, "nsblk": jS - sblk0,
            "bank_segs": [(int(gb_blk0[g, b] - gb_blk0[g, 0]),
                           int(gb_nblk[g, b])) for b in range(NBANKS)],
            "tile_mms": tile_mms,
        })
    tot_sblk = jS

    # per-core streams
    per_core = []
    for c in range(NCORES):
        d_l, s_l, w_l, tile, bank = cores[c]
        ne = len(d_l)
        # per-edge slot: R_tb[t, b] + rank within its (t, b) segment.
        # edges are already sorted in segment-iteration order.
        seg_sizes_iter = []
        seg_R_iter = []
        for g in range(n_groups):
            for b in range(NBANKS):
                for t in group_tiles[g]:
                    seg_sizes_iter.append(counts_all[c, t, b])
                    seg_R_iter.append(R_tb[t, b])
        seg_sizes_iter = np.array(seg_sizes_iter, np.int64)
        seg_R_iter = np.array(seg_R_iter, np.int64)
        first = np.concatenate([[0], np.cumsum(seg_sizes_iter)])[:-1]
        pos = (np.repeat(seg_R_iter, seg_sizes_iter)
               + np.arange(ne) - np.repeat(first, seg_sizes_iter))

        # gather idx stream: real rows at their slots; pad slots repeat the
        # previous real row (forward fill).
        idx_stream = np.zeros(tot_slots, np.int16)
        idx_stream[pos] = (s_l % BROWS).astype(np.int16)
        valid = np.zeros(tot_slots, bool)
        valid[pos] = True
        vi = np.where(valid, np.arange(tot_slots), 0)
        np.maximum.accumulate(vi, out=vi)
        idx_stream = idx_stream[vi]

        idx16 = np.ascontiguousarray(
            np.tile(idx_stream.reshape(-1, 16).T, (8, 1)))   # [128, totblk*8]

        # S stream: s_flat[jS*128 + lane, dl] = w for edges whose G-slot
        # falls in (t, b, jG); pad/foreign slots stay zero.
        jG_e = pos // CHUNK
        lane_e = pos % CHUNK
        # map (t, b, jG) -> jS: build lookup per (t, b) over its block span
        jS_e = np.empty(ne, np.int64)
        # vectorized via dict over segments (396 entries)
        tb_of_edge = tile * NBANKS + bank
        s_lut = {}
        for (jSv, jGv, tv, bv) in s_entries:
            s_lut[(tv * NBANKS + bv, jGv)] = jSv
        # per-edge lookup: loop over segments, slice by sorted order
        seg_ids = tb_of_edge
        # edges grouped by segment contiguously; find boundaries
        segb = np.flatnonzero(np.diff(seg_ids)) + 1
        starts = np.concatenate([[0], segb])
        ends = np.concatenate([segb, [ne]])
        for s0, e0 in zip(starts, ends):
            key = int(seg_ids[s0])
            jSs = np.array([s_lut[(key, int(jg))] for jg in
                            np.unique(jG_e[s0:e0])], np.int64)
            uj, inv = np.unique(jG_e[s0:e0], return_inverse=True)
            jS_e[s0:e0] = jSs[inv]

        s_flat = np.zeros((tot_sblk * CHUNK, TILE_W), np.float32)
        s_flat[jS_e * CHUNK + lane_e, d_l] = w_l
        s_np = np.ascontiguousarray(
            s_flat.astype(bf16).reshape(tot_sblk, CHUNK, TILE_W)
            .transpose(1, 0, 2))                             # [128, sblk, T]
        per_core.append({"idx16": idx16, "s": s_np})

    xb = np.asarray(x, np.float32).astype(bf16)  # [N, D]
    banks = [np.ascontiguousarray(xb[b * BROWS:(b + 1) * BROWS])
             for b in range(NBANKS)]

    return per_core, banks, groups, totblk, tot_sblk, core_of, col_of


def _build_program(groups, totblk, tot_sblk):
    import concourse.bass as bass  # noqa: F401
    import concourse.bacc as bacc
    import concourse.mybir as mybir
    import concourse.tile as tile
    from concourse import library_config

    nc = bacc.Bacc("TRN2", target_bir_lowering=False, debug=False,
                   num_devices=NCORES, num_swdge_queues=NQUEUES)

    xb_d = [nc.dram_tensor(f"xb{b}", [BROWS, D], mybir.dt.bfloat16,
                           kind="ExternalInput") for b in range(NBANKS)]
    idx_d = nc.dram_tensor("idx16", [128, totblk * 8], mybir.dt.int16,
                           kind="ExternalInput")
    s_d = nc.dram_tensor("s", [128, tot_sblk, TILE_W], mybir.dt.bfloat16,
                         kind="ExternalInput")
    wt_d = nc.dram_tensor("wt", [D, D], mybir.dt.float32, kind="ExternalInput")
    b_d = nc.dram_tensor("b", [D, 1], mybir.dt.float32, kind="ExternalInput")
    out_d = nc.dram_tensor("outT", [D, T_TILES * TILE_W], mybir.dt.float32,
                           kind="ExternalOutput")

    max_nblk = max(g["nblk"] for g in groups)
    max_nsblk = max(g["nsblk"] for g in groups)
    SUP_W = GT * TILE_W  # supertile width (384)

    with tile.TileContext(nc) as tc:
        with (
            tc.tile_pool(name="const", bufs=1) as constp,
            tc.tile_pool(name="meta", bufs=8) as metap,
            tc.tile_pool(name="gather", bufs=3) as gatherp,
            tc.tile_pool(name="s", bufs=2) as sp,
            tc.tile_pool(name="agg", bufs=2) as aggp,
            tc.tile_pool(name="outp", bufs=2) as outp,
            tc.tile_pool(name="psum", bufs=6, space="PSUM") as psump,
            tc.tile_pool(name="psum2", bufs=2, space="PSUM") as psum2p,
        ):
            nc.gpsimd.load_library(library_config.mlp)

            wt_t = constp.tile([D, D], mybir.dt.float32)
            b_t = constp.tile([D, 1], mybir.dt.float32)
            nc.sync.dma_start(wt_t[:], wt_d[:])
            nc.sync.dma_start(b_t[:], b_d[:])

            for gi, g in enumerate(groups):
                blk0, nblk_g = g["blk0"], g["nblk"]
                sblk0, nsblk_g = g["sblk0"], g["nsblk"]
                ntile_g = len(g["tile_mms"])
                sup_w = ntile_g * TILE_W
                idx_t = metap.tile([128, max_nblk * 8], mybir.dt.int16,
                                   tag="idx")
                nc.sync.dma_start(idx_t[:, :nblk_g * 8],
                                  idx_d[:, blk0 * 8:(blk0 + nblk_g) * 8])

                s_t = sp.tile([128, max_nsblk, TILE_W], mybir.dt.bfloat16,
                              tag="s")
                nc.sync.dma_start(s_t[:, :nsblk_g, :],
                                  s_d[:, sblk0:sblk0 + nsblk_g, :])

                g_t = gatherp.tile([128, max_nblk, D], mybir.dt.bfloat16,
                                   tag="g")
                # One gather call is capped at 1024 indices (8 blocks) --
                # larger num_idxs hangs the ucode on hardware. Bank b's
                # calls go to SWDGE queue b (4 Q7 core pairs in parallel),
                # issued bank-interleaved so the 8-deep GpSimd engine queue
                # always spans all 4 queues. GpSimd retires in order, so
                # each 4-queue "wave" advances at the pace of its largest
                # call: split every bank into the same number of near-equal
                # calls so no wave is ragged.
                MAXG = 8
                ncalls = max((g["bank_segs"][b][1] + MAXG - 1) // MAXG
                             for b in range(NBANKS))
                call_lists = []  # per bank: list of (a, ln)
                for b in range(NBANKS):
                    boff, blen = g["bank_segs"][b]
                    base, rem = divmod(blen, ncalls)
                    calls = []
                    a = boff
                    for ci in range(ncalls):
                        ln = base + (1 if ci < rem else 0)
                        if ln:
                            calls.append((a, ln))
                            a += ln
                    call_lists.append(calls)
                for ci in range(ncalls):
                    for b in range(NBANKS):
                        if ci >= len(call_lists[b]):
                            continue
                        a, ln = call_lists[b][ci]
                        nc.gpsimd.dma_gather(
                            g_t[:, a:a + ln, :],
                            xb_d[b][:],
                            idx_t[:, a * 8:(a + ln) * 8],
                            ln * CHUNK,
                            ln * CHUNK,
                            D,
                            queue_num=b,
                        )

                agg4_t = aggp.tile([D, SUP_W], mybir.dt.float32, tag="agg")
                for ti, (t, mms) in enumerate(g["tile_mms"]):
                    assert mms, f"tile {t} has no edge blocks"
                    psum_t = psump.tile([D, TILE_W], mybir.dt.float32,
                                        tag="p1")
                    for k, (jG, jS) in enumerate(mms):
                        nc.tensor.matmul(
                            psum_t[:], g_t[:, jG, :], s_t[:, jS, :],
                            start=(k == 0), stop=(k == len(mms) - 1),
                        )

                    nc.scalar.copy(
                        agg4_t[:, ti * TILE_W:(ti + 1) * TILE_W], psum_t[:])

                psum2_t = psum2p.tile([D, SUP_W], mybir.dt.float32, tag="p2")
                nc.tensor.matmul(psum2_t[:, :sup_w], wt_t[:],
                                 agg4_t[:, :sup_w], start=True, stop=True)

                out_t = outp.tile([D, SUP_W], mybir.dt.float32, tag="o")
                nc.scalar.activation(
                    out_t[:, :sup_w], psum2_t[:, :sup_w],
                    mybir.ActivationFunctionType.Identity,
                    bias=b_t[:, 0:1], scale=1.0,
                )
                t0 = g["tile_mms"][0][0]
                nc.sync.dma_start(
                    out_d[:, t0 * TILE_W:t0 * TILE_W + sup_w],
                    out_t[:, :sup_w])

    nc.compile()
    return nc


LAST_RES = None


def kernel(x, edge_index, edge_weight, W, b):
    import os
    from concourse.bass_utils import run_bass_kernel_spmd

    per_core, banks, groups, totblk, tot_sblk, core_of, col_of = _host_prep(
        x, edge_index, edge_weight)

    nc = _build_program(groups, totblk, tot_sblk)

    WT = np.ascontiguousarray(np.asarray(W, np.float32).T)  # [din, dout]
    bcol = np.ascontiguousarray(np.asarray(b, np.float32).reshape(D, 1))

    in_maps = []
    for c in range(NCORES):
        p = per_core[c]
        m = {f"xb{i}": banks[i] for i in range(NBANKS)}
        m.update({
            "idx16": p["idx16"], "s": p["s"], "wt": WT, "b": bcol,
        })
        in_maps.append(m)

    res = run_bass_kernel_spmd(
        nc, in_maps, core_ids=list(range(NCORES)),
        trace=bool(int(os.environ.get("KERNEL_TRACE", "0"))),
    )
    global LAST_RES
    LAST_RES = res

    out = np.empty((N_NODES, D), np.float32)
    for c in range(NCORES):
        outT = res.results[c]["outT"]  # [D, T*TILE_W]
        dsts_c = np.flatnonzero(core_of == c)
        out[dsts_c] = outT[:, col_of[dsts_c]].T
    return out


if __name__ == "__main__":
    # smoke test with random data (self-contained)
    rng = np.random.default_rng(0)
    x = rng.standard_normal((N_NODES, D)).astype(np.float32)
    ei = rng.integers(0, N_NODES, size=(2, N_EDGES)).astype(np.int64)
    ew = rng.random(N_EDGES).astype(np.float32)
    W = (rng.standard_normal((D, D)) / np.sqrt(D)).astype(np.float32)
    b = (rng.standard_normal(D) * 0.01).astype(np.float32)
    out = kernel(x, ei, ew, W, b)
    print("out", out.shape, out.dtype)
